# revision 16
# baseline (speedup 1.0000x reference)
"""Trainium2 Bass kernel for nn_EstimatorQNN (MLP -> pairwise fidelity graph -> adj @ out).

Contract: kernel(**inputs) takes FULL unsharded inputs (numpy, fp32) and
returns the FULL [8192, 64] fp32 output.  Internally the batch is sharded
across 8 NeuronCores (data parallel); the [8192, 64] MLP outputs are gathered
and normalized on host between the two device launches.

Phase 1 (per core): outT = MLP(x_local).T  (pure MLP; batched DMA inputs)
Host:               + b3, normalize (fp32), build fp8 DoubleRow layouts
Phase 2 (per core): per m-block b: fid[128, 1024] = nf_b^T @ nl (fp8 DoubleRow,
                    4-way row-tiled); mask = (|fid| >= sqrt(0.9)) in ONE
                    tensor_scalar op (abs_max then is_ge) spread across
                    DVE/Pool/ACT; local-block diagonals forced to 1 via
                    affine_select; y[j, d] += mask[:, jc]^T @ out_b (reversed
                    orientation, N=64) accumulated in one PSUM bank over all
                    64 blocks; finally y -= out_local (removes the forced
                    diagonal) and one DMA out.
"""

import contextlib

import numpy as np
import ml_dtypes

import concourse.bass as bass
import concourse.tile as tile
from concourse import bacc, mybir
from concourse.bass_utils import run_bass_kernel_spmd
from concourse.bass_interp import get_hw_module

F32 = mybir.dt.float32
BF16 = mybir.dt.bfloat16
FP8 = mybir.dt.float8e4
AF = mybir.ActivationFunctionType
ALU = mybir.AluOpType
PM = mybir.MatmulPerfMode

B, D_IN, H1, H2, D_OUT = 8192, 256, 512, 256, 64
NCORES = 8
LOCAL = B // NCORES          # 1024 rows per core
THRESHOLD = 0.9
SQRT_T = float(np.sqrt(np.float32(THRESHOLD)))   # |dot| >= sqrt(T)  <=>  dot^2 >= T


# ---------------------------------------------------------------------------
# Phase 1: pure MLP.  Inputs (per core):
#   xT  [128, 2, 1024] bf16   xT[p, kc, j] = x_local[j, kc*128 + p]
#   wpk [128, 2176] bf16      w1 [128,2,512] | w2 [128,4,256] | w3 [128,2,64]
#   bpk [128, 6] f32          b1 [128,4] | b2 [128,2]
# Output: outT [64, 1024] f32   (b3 added on host)
# ---------------------------------------------------------------------------
def build_phase1(n_b=LOCAL, reps=1, loop_reps=1):
    nb = n_b // 512          # number of 512-wide batch chunks
    nc = bacc.Bacc("TRN2", target_bir_lowering=False, debug=False,
                   enable_asserts=False, num_devices=NCORES)
    xT = nc.dram_tensor("xT", [128, 2, n_b], BF16, kind="ExternalInput")
    wpk = nc.dram_tensor("wpk", [128, 2176], BF16, kind="ExternalInput")
    bpk = nc.dram_tensor("bpk", [128, 6], F32, kind="ExternalInput")
    outT = nc.dram_tensor("outT", [64, n_b], F32, kind="ExternalOutput")

    with tile.TileContext(nc) as tc:
        with (
            tc.tile_pool(name="wpool", bufs=1) as wpool,
            tc.tile_pool(name="hpool", bufs=1) as hpool,
            tc.tile_pool(name="outp", bufs=2) as outp,
            tc.tile_pool(name="ph1", bufs=2, space="PSUM") as ph1,
            tc.tile_pool(name="ph2", bufs=2, space="PSUM") as ph2p,
            tc.tile_pool(name="ps64", bufs=2, space="PSUM") as ps64,
            (tc.For_i(0, loop_reps) if loop_reps > 1
             else contextlib.nullcontext()),
        ):
            for rep in range(reps):
                x_sb = wpool.tile([128, 2, n_b], BF16, tag="x")
                w_sb = wpool.tile([128, 2176], BF16, tag="w")
                b_sb = wpool.tile([128, 6], F32, tag="b")
                # critical path: w1 + first x chunk on the sync queue
                nc.sync.dma_start(w_sb[:, 0:1024], wpk[:, 0:1024])
                nc.sync.dma_start(x_sb[:, :, 0:512], xT[:, :, 0:512])
                if n_b > 512:
                    nc.sync.dma_start(x_sb[:, :, 512:n_b], xT[:, :, 512:n_b])
                # later-stage weights ride the gpsimd-triggered queue
                nc.gpsimd.dma_start(w_sb[:, 1024:2176], wpk[:, 1024:2176])
                nc.gpsimd.dma_start(b_sb[:], bpk[:])

                h1_sb = hpool.tile([128, 4, n_b], BF16, tag="h1")
                h2_sb = hpool.tile([128, 2, n_b], BF16, tag="h2")

                for bb in range(nb):
                    sl = slice(bb * 512, (bb + 1) * 512)
                    ph2_0 = ph2p.tile([128, 512], F32, tag="h2_0")
                    ph2_1 = ph2p.tile([128, 512], F32, tag="h2_1")
                    ph2 = [ph2_0, ph2_1]
                    for hb in range(4):
                        ps = ph1.tile([128, 512], F32, tag="h1")
                        for kc in range(2):
                            w1s = slice(kc * 512 + hb * 128,
                                        kc * 512 + (hb + 1) * 128)
                            nc.tensor.matmul(
                                ps[:], w_sb[:, w1s], x_sb[:, kc, sl],
                                start=(kc == 0), stop=(kc == 1))
                        nc.scalar.activation(
                            h1_sb[:, hb, sl], ps[:],
                            AF.Tanh, bias=b_sb[:, hb:hb + 1], scale=1.0)
                        for hb2 in range(2):
                            w2s = slice(1024 + hb * 256 + hb2 * 128,
                                        1024 + hb * 256 + (hb2 + 1) * 128)
                            nc.tensor.matmul(
                                ph2[hb2][:], w_sb[:, w2s], h1_sb[:, hb, sl],
                                start=(hb == 0), stop=(hb == 3))
                    pso = ps64.tile([64, 512], F32, tag="mmo")
                    for hb2 in range(2):
                        nc.scalar.activation(
                            h2_sb[:, hb2, sl], ph2[hb2][:],
                            AF.Tanh, bias=b_sb[:, 4 + hb2:5 + hb2], scale=1.0)
                        w3s = slice(2048 + hb2 * 64, 2048 + (hb2 + 1) * 64)
                        nc.tensor.matmul(
                            pso[:], w_sb[:, w3s], h2_sb[:, hb2, sl],
                            start=(hb2 == 0), stop=(hb2 == 1))
                    # split evacuation so the output DMA streams early
                    for half in range(2):
                        osl = slice(bb * 512 + half * 256,
                                    bb * 512 + half * 256 + 256)
                        psl = slice(half * 256, half * 256 + 256)
                        ot = outp.tile([64, 256], F32, tag="oc")
                        nc.vector.tensor_copy(ot[:], pso[:, psl])
                        nc.sync.dma_start(outT[:, osl], ot[:])

    nc.compile()
    return nc


# ---------------------------------------------------------------------------
# Phase 2: gram (fp8 DoubleRow) + one-op threshold + adjacency matmul.
# Inputs (per core, batch rolled by -1024*core so blocks 0..7 are local):
#   nf8 [128, 16, 2, 128] fp8 -- normedT_full in DoubleRow layout, block b=4g+t
#       at partitions 32t..32t+32 of group g:
#       nf8[32t+ki, g, ko, m] = normed[d = ki + 32*ko, row 128*b + m]
#   nl8 [128, 2, 1024] fp8 -- this core's normedT slice, replicated per
#       row-tile group: nl8[32t+ki, ko, j] = normed_local[ki + 32*ko, j]
#   obf [128, 64, 64] bf16 -- out_full rolled; [p, b, d] = out[b*128+p, d]
#   olt [64, 1024] bf16 -- out_local^T (for the diagonal subtraction)
# Output: yt [64, 1024] f32 -- yT for the local rows
# ---------------------------------------------------------------------------
def build_phase2(n_kb=B // 128, reps=1, loop_reps=1, lag=6,
                 n_a=4, n_p=41, mask_bufs=None, fid_bufs=3):
    ngrp = n_kb // 4
    nloc = LOCAL // 128      # local blocks (with a diagonal) = 8
    nc = bacc.Bacc("TRN2", target_bir_lowering=False, debug=False,
                   enable_asserts=False, num_devices=NCORES)
    nf8 = nc.dram_tensor("nf8", [128, ngrp, 2, 128], FP8, kind="ExternalInput")
    nl8 = nc.dram_tensor("nl8", [128, 2, LOCAL], FP8, kind="ExternalInput")
    obf = nc.dram_tensor("obf", [128, n_kb, 64], BF16, kind="ExternalInput")
    olt = nc.dram_tensor("olt", [64, LOCAL], BF16, kind="ExternalInput")
    yt = nc.dram_tensor("yt", [64, LOCAL], F32, kind="ExternalOutput")

    # Engine assignment: n_a tiles ACT+DVE, n_p tiles ACT+Pool, rest DVE
    # two-sided (unmerged, 2x yT).  Local blocks (b < nloc) must be mergeable
    # for affine_select, so they never take the D path.
    n_d = n_kb - n_a - n_p
    assign = []
    cnt = {"A": 0, "P": 0, "D": 0}
    quota = {"A": n_a, "P": n_p, "D": n_d}
    for b in range(n_kb):
        order = (["P", "A", "D"] if b < nloc else ["D", "P", "A"])
        # round-robin proportional fill
        best = max(order, key=lambda k: (quota[k] - cnt[k]) / max(quota[k], 1))
        if b < nloc and best == "D":
            best = "P" if (quota["P"] - cnt["P"]) >= (quota["A"] - cnt["A"]) \
                else "A"
        assign.append(best)
        cnt[best] += 1

    if mask_bufs is None:
        mask_bufs = 2 * lag + 4
    with tile.TileContext(nc) as tc:
        with (
            tc.tile_pool(name="big", bufs=1) as big,
            tc.tile_pool(name="maskp", bufs=mask_bufs) as maskp,
            tc.tile_pool(name="absp", bufs=4) as absp,
            tc.tile_pool(name="outp", bufs=2) as outp,
            tc.tile_pool(name="fidp", bufs=fid_bufs, space="PSUM") as fidp,
            tc.tile_pool(name="ytp", bufs=1, space="PSUM") as ytp,
            (tc.For_i(0, loop_reps) if loop_reps > 1
             else contextlib.nullcontext()),
        ):
            for rep in range(reps):
                nf_sb = big.tile([128, ngrp, 2, 128], FP8, tag="nf")
                nl_sb = big.tile([128, 2, LOCAL], FP8, tag="nl")
                ob_sb = big.tile([128, n_kb, 64], BF16, tag="ob")
                ol_sb = big.tile([64, LOCAL], BF16, tag="ol")
                # critical-path order: local cols + first nf groups so fid
                # starts ASAP; first ob blocks before the yT lag expires
                nc.sync.dma_start(nl_sb[:], nl8[:])
                nc.sync.dma_start(nf_sb[:, 0:4], nf8[:, 0:4])
                nc.sync.dma_start(ob_sb[:, 0:16], obf[:, 0:16, :])
                nc.gpsimd.dma_start(nf_sb[:, 4:ngrp], nf8[:, 4:ngrp])
                nc.gpsimd.dma_start(ob_sb[:, 16:n_kb], obf[:, 16:n_kb, :])
                nc.gpsimd.dma_start(ol_sb[:], olt[:])

                yps = ytp.tile([128, 512], F32, tag="y")

                mq = {}
                for step in range(n_kb + lag):
                    if step < n_kb:
                        b = step
                        g, t = b // 4, b % 4
                        sl32 = slice(32 * t, 32 * t + 32)
                        fps = fidp.tile([128, 1024], F32, tag="fid")
                        for half in range(2):
                            nsl = slice(half * 512, (half + 1) * 512)
                            nc.tensor.matmul(
                                fps[:, nsl], nf_sb[sl32, g, :, :],
                                nl_sb[sl32, :, nsl],
                                start=True, stop=True,
                                perf_mode=PM.DoubleRow,
                                tile_position=(32 * t, 0))
                        path = assign[b]
                        if path == "D":
                            # two one-sided compares straight from PSUM;
                            # left unmerged (yT consumes both: or == add)
                            g1 = maskp.tile([128, 1024], BF16, tag="mask")
                            g2 = maskp.tile([128, 1024], BF16, tag="mask")
                            nc.vector.tensor_scalar(
                                g1[:], fps[:], SQRT_T, None, op0=ALU.is_ge)
                            nc.vector.tensor_scalar(
                                g2[:], fps[:], -SQRT_T, None, op0=ALU.is_le)
                            mq[step] = (g1, g2)
                        else:
                            mask = maskp.tile([128, 1024], BF16, tag="mask")
                            ab = absp.tile([128, 1024], BF16, tag="abs")
                            nc.scalar.activation(ab[:], fps[:], AF.Abs)
                            eng = nc.gpsimd if path == "P" else nc.vector
                            eng.tensor_scalar(
                                mask[:], ab[:], SQRT_T, None, op0=ALU.is_ge)
                            if b < nloc:
                                # force the diagonal of local block b to 1
                                # (iota = j - p - 128*b; keep where != 0)
                                nc.gpsimd.affine_select(
                                    mask[:], mask[:], pattern=[[1, 1024]],
                                    compare_op=ALU.not_equal, fill=1.0,
                                    base=-128 * b, channel_multiplier=-1)
                            mq[step] = (mask,)
                    if step >= lag:
                        b2 = step - lag
                        parts = mq.pop(b2)
                        for gi, gt in enumerate(parts):
                            first = (b2 == 0) and gi == 0
                            last = (b2 == n_kb - 1) and gi == len(parts) - 1
                            nc.tensor.matmul(
                                yps[0:64, :], ob_sb[:, b2, :],
                                gt[:, 0:512],
                                start=first, stop=last,
                                tile_position=(0, 0))
                            nc.tensor.matmul(
                                yps[64:128, :], ob_sb[:, b2, :],
                                gt[:, 512:1024],
                                start=first, stop=last,
                                tile_position=(0, 64))

                # yT -= out_local^T (removes the forced diagonal), then DMA
                for half in range(2):
                    jsl = slice(half * 512, (half + 1) * 512)
                    ysb = outp.tile([64, 512], F32, tag="ys")
                    nc.vector.tensor_tensor(
                        ysb[:], yps[half * 64:(half + 1) * 64, :],
                        ol_sb[:, jsl], op=ALU.subtract)
                    nc.sync.dma_start(yt[:, jsl], ysb[:])

    nc.compile()
    return nc


# ---------------------------------------------------------------------------
# Host orchestration
# ---------------------------------------------------------------------------
_CACHE = {}
LAST_RESULTS = {}


def _get(name, builder):
    if name not in _CACHE:
        nc = builder()
        nc.m = get_hw_module(nc.m)
        _CACHE[name] = nc
    return _CACHE[name]


def _phase1_inmaps(x, W1, b1, W2, b2, W3, b3):
    bf = ml_dtypes.bfloat16
    w1v = np.ascontiguousarray(
        W1.T.reshape(2, 128, 512).transpose(1, 0, 2)).reshape(128, 1024)
    w2v = np.ascontiguousarray(
        W2.T.reshape(4, 128, 256).transpose(1, 0, 2)).reshape(128, 1024)
    w3v = np.ascontiguousarray(
        W3.T.reshape(2, 128, 64).transpose(1, 0, 2)).reshape(128, 128)
    wpk = np.concatenate([w1v, w2v, w3v], axis=1).astype(bf)
    bpk = np.concatenate(
        [b1.reshape(4, 128).T, b2.reshape(2, 128).T], axis=1
    ).astype(np.float32)
    bpk = np.ascontiguousarray(bpk)
    maps = []
    for c in range(NCORES):
        xT = np.ascontiguousarray(
            x[c * LOCAL:(c + 1) * LOCAL].T.reshape(2, 128, LOCAL)
            .transpose(1, 0, 2)).astype(bf)
        maps.append(dict(xT=xT, wpk=wpk, bpk=bpk))
    return maps


def _phase2_inmaps(normedT_full, out_full):
    fp8 = ml_dtypes.float8_e4m3
    out_bf = out_full.astype(ml_dtypes.bfloat16)
    n_kb = B // 128
    ngrp = n_kb // 4
    maps = []
    for c in range(NCORES):
        nfull = np.roll(normedT_full, -LOCAL * c, axis=1)   # [64, 8192] f32
        # nf8[32t+ki, g, ko, m] = nfull[d = ki + 32*ko, row 128*(4g+t) + m]
        blk = nfull.reshape(2, 32, ngrp, 4, 128)            # [ko, ki, g, t, m]
        nf8 = np.ascontiguousarray(
            blk.transpose(3, 1, 2, 0, 4).reshape(128, ngrp, 2, 128)).astype(fp8)
        nloc = normedT_full[:, c * LOCAL:(c + 1) * LOCAL]   # [64, 1024] f32
        # nl8[32t+ki, ko, j] = nloc[ki + 32*ko, j], replicated over t
        nl1 = np.ascontiguousarray(
            nloc.reshape(2, 32, LOCAL).transpose(1, 0, 2))  # [ki, ko, j]
        nl8 = np.ascontiguousarray(
            np.broadcast_to(nl1, (4, 32, 2, LOCAL)).reshape(128, 2, LOCAL)
        ).astype(fp8)
        ob = np.roll(out_bf, -LOCAL * c, axis=0)
        olt = np.ascontiguousarray(ob[0:LOCAL].T)           # [64, 1024] bf16
        ob = np.ascontiguousarray(ob.reshape(64, 128, 64).transpose(1, 0, 2))
        maps.append(dict(nf8=nf8, nl8=nl8, obf=ob, olt=olt))
    return maps


def kernel(x, W1, b1, W2, b2, W3, b3, _trace=False):
    x, W1, b1, W2, b2, W3, b3 = [
        np.asarray(a, dtype=np.float32) for a in (x, W1, b1, W2, b2, W3, b3)]
    nc1 = _get("p1", build_phase1)
    nc2 = _get("p2", build_phase2)

    r1 = run_bass_kernel_spmd(nc1, _phase1_inmaps(x, W1, b1, W2, b2, W3, b3),
                              core_ids=list(range(NCORES)), trace=_trace)
    outT_full = np.concatenate([r1.results[c]["outT"] for c in range(NCORES)],
                               axis=1)
    outT_full = outT_full + b3[:, None].astype(np.float32)  # bias on host
    out_full = np.ascontiguousarray(outT_full.T)            # [8192, 64] f32
    norms = np.linalg.norm(out_full, axis=-1, keepdims=True)
    normed = out_full / (norms + np.float32(1e-12))
    normedT_full = np.ascontiguousarray(normed.T.astype(np.float32))

    r2 = run_bass_kernel_spmd(nc2, _phase2_inmaps(normedT_full, out_full),
                              core_ids=list(range(NCORES)), trace=_trace)
    y = np.concatenate(
        [np.ascontiguousarray(r2.results[c]["yt"].T) for c in range(NCORES)],
        axis=0)
    LAST_RESULTS["r1"] = r1
    LAST_RESULTS["r2"] = r2
    return y.astype(np.float32)


# revision 47
# speedup vs baseline: 2.0933x; 2.0933x over previous
"""Trainium2 Bass kernel for nn_EstimatorQNN (MLP -> pairwise fidelity graph -> adj @ out).

Contract: kernel(**inputs) takes FULL unsharded inputs (numpy, fp32) and
returns the FULL [8192, 64] fp32 output.  Internally the batch is sharded
across 8 NeuronCores (data parallel); the [8192, 64] MLP outputs are gathered
and normalized on host between the two device launches.

Phase 1 (per core): outT = MLP(x_local).T  (pure MLP; batched DMA inputs)
Host:               + b3, normalize (fp32), build fp8 DoubleRow layouts
Phase 2 (per core): per m-block b: fid[128, 1024] = nf_b^T @ nl (fp8 DoubleRow,
                    4-way row-tiled); mask = (|fid| >= sqrt(0.9)) in ONE
                    tensor_scalar op (abs_max then is_ge) spread across
                    DVE/Pool/ACT; local-block diagonals forced to 1 via
                    affine_select; y[j, d] += mask[:, jc]^T @ out_b (reversed
                    orientation, N=64) accumulated in one PSUM bank over all
                    64 blocks; finally y -= out_local (removes the forced
                    diagonal) and one DMA out.
"""

import contextlib

import numpy as np
import ml_dtypes

import concourse.bass as bass
import concourse.tile as tile
from concourse import bacc, mybir
from concourse.bass_utils import run_bass_kernel_spmd
from concourse.bass_interp import get_hw_module

F32 = mybir.dt.float32
BF16 = mybir.dt.bfloat16
FP8 = mybir.dt.float8e4
AF = mybir.ActivationFunctionType
ALU = mybir.AluOpType
PM = mybir.MatmulPerfMode

B, D_IN, H1, H2, D_OUT = 8192, 256, 512, 256, 64
NCORES = 8
LOCAL = B // NCORES          # 1024 rows per core
THRESHOLD = 0.9
SQRT_T = float(np.sqrt(np.float32(THRESHOLD)))   # |dot| >= sqrt(T)  <=>  dot^2 >= T


# ---------------------------------------------------------------------------
# Phase 1: pure MLP.  Inputs (per core):
#   xT  [128, 2, 1024] bf16   xT[p, kc, j] = x_local[j, kc*128 + p]
#   wpk [128, 2176] bf16      w1 [128,2,512] | w2 [128,4,256] | w3 [128,2,64]
#   bpk [128, 6] f32          b1 [128,4] | b2 [128,2]
# Output: outT [64, 1024] f32   (b3 added on host)
# ---------------------------------------------------------------------------
def build_phase1(n_b=LOCAL, reps=1, loop_reps=1):
    nb = n_b // 512          # number of 512-wide batch chunks
    nc = bacc.Bacc("TRN2", target_bir_lowering=False, debug=False,
                   enable_asserts=False, num_devices=NCORES)
    xT = nc.dram_tensor("xT", [128, 2, n_b], BF16, kind="ExternalInput")
    wpk = nc.dram_tensor("wpk", [128, 2176], BF16, kind="ExternalInput")
    bpk = nc.dram_tensor("bpk", [128, 6], F32, kind="ExternalInput")
    outT = nc.dram_tensor("outT", [64, n_b], F32, kind="ExternalOutput")

    with tile.TileContext(nc) as tc:
        with (
            tc.tile_pool(name="wpool", bufs=1) as wpool,
            tc.tile_pool(name="hpool", bufs=1) as hpool,
            tc.tile_pool(name="outp", bufs=2) as outp,
            tc.tile_pool(name="ph1", bufs=3, space="PSUM") as ph1,
            tc.tile_pool(name="ph2", bufs=2, space="PSUM") as ph2p,
            tc.tile_pool(name="ps64", bufs=1, space="PSUM") as ps64,
            (tc.For_i(0, loop_reps) if loop_reps > 1
             else contextlib.nullcontext()),
        ):
            for rep in range(reps):
                x_sb = wpool.tile([128, 2, n_b], BF16, tag="x")
                w_sb = wpool.tile([128, 2176], BF16, tag="w")
                b_sb = wpool.tile([128, 6], F32, tag="b")
                # critical path: w1 + first x chunk on the sync queue
                nc.sync.dma_start(w_sb[:, 0:1024], wpk[:, 0:1024])
                nc.sync.dma_start(x_sb[:, :, 0:512], xT[:, :, 0:512])
                if n_b > 512:
                    nc.sync.dma_start(x_sb[:, :, 512:n_b], xT[:, :, 512:n_b])
                # later-stage weights ride the gpsimd-triggered queue
                nc.gpsimd.dma_start(w_sb[:, 1024:2176], wpk[:, 1024:2176])
                nc.gpsimd.dma_start(b_sb[:], bpk[:])

                h1_sb = hpool.tile([128, 4, n_b], BF16, tag="h1")
                h2_sb = hpool.tile([128, 2, n_b], BF16, tag="h2")

                for bb in range(nb):
                    sl = slice(bb * 512, (bb + 1) * 512)
                    ph2_0 = ph2p.tile([128, 512], F32, tag="h2_0")
                    ph2_1 = ph2p.tile([128, 512], F32, tag="h2_1")
                    ph2 = [ph2_0, ph2_1]
                    for hb in range(4):
                        ps = ph1.tile([128, 512], F32, tag="h1")
                        for kc in range(2):
                            w1s = slice(kc * 512 + hb * 128,
                                        kc * 512 + (hb + 1) * 128)
                            nc.tensor.matmul(
                                ps[:], w_sb[:, w1s], x_sb[:, kc, sl],
                                start=(kc == 0), stop=(kc == 1))
                        nc.scalar.activation(
                            h1_sb[:, hb, sl], ps[:],
                            AF.Tanh, bias=b_sb[:, hb:hb + 1], scale=1.0)
                        for hb2 in range(2):
                            w2s = slice(1024 + hb * 256 + hb2 * 128,
                                        1024 + hb * 256 + (hb2 + 1) * 128)
                            nc.tensor.matmul(
                                ph2[hb2][:], w_sb[:, w2s], h1_sb[:, hb, sl],
                                start=(hb == 0), stop=(hb == 3))
                    pso = ps64.tile([64, 512], F32, tag="mmo")
                    for hb2 in range(2):
                        nc.scalar.activation(
                            h2_sb[:, hb2, sl], ph2[hb2][:],
                            AF.Tanh, bias=b_sb[:, 4 + hb2:5 + hb2], scale=1.0)
                        w3s = slice(2048 + hb2 * 64, 2048 + (hb2 + 1) * 64)
                        nc.tensor.matmul(
                            pso[:], w_sb[:, w3s], h2_sb[:, hb2, sl],
                            start=(hb2 == 0), stop=(hb2 == 1))
                    osl = slice(bb * 512, (bb + 1) * 512)
                    ot = outp.tile([64, 512], F32, tag="oc")
                    nc.vector.tensor_copy(ot[:], pso[:])
                    nc.sync.dma_start(outT[:, osl], ot[:])

    nc.compile()
    return nc


# ---------------------------------------------------------------------------
# Phase 2: gram (fp8 DoubleRow) + one-op threshold + adjacency matmul.
# Inputs (per core, batch rolled by -1024*core so blocks 0..7 are local):
#   nf8 [128, 16, 2, 128] fp8 -- normedT_full in DoubleRow layout, block b=4g+t
#       at partitions 32t..32t+32 of group g:
#       nf8[32t+ki, g, ko, m] = normed[d = ki + 32*ko, row 128*b + m]
#   nl8 [128, 2, 1024] fp8 -- this core's normedT slice, replicated per
#       row-tile group: nl8[32t+ki, ko, j] = normed_local[ki + 32*ko, j]
#   obf [128, 64, 64] bf16 -- out_full rolled; [p, b, d] = out[b*128+p, d]
#   olt [64, 1024] bf16 -- out_local^T (for the diagonal subtraction)
# Output: yt [64, 1024] f32 -- yT for the local rows
# ---------------------------------------------------------------------------
def build_phase2(n_kb=B // 128, reps=1, loop_reps=1, lag=6,
                 n_a=44, n_p=10, mask_bufs=None, fid_bufs=3, no_affine=False,
                 mode="full"):
    ngrp = n_kb // 4
    nloc = LOCAL // 128      # local blocks (with a diagonal) = 8
    nc = bacc.Bacc("TRN2", target_bir_lowering=False, debug=False,
                   enable_asserts=False, num_devices=NCORES)
    nf8 = nc.dram_tensor("nf8", [128, ngrp, 2, 128], FP8, kind="ExternalInput")
    nl8 = nc.dram_tensor("nl8", [128, 2, LOCAL], FP8, kind="ExternalInput")
    obf = nc.dram_tensor("obf", [128, n_kb, 64], BF16, kind="ExternalInput")
    olt = nc.dram_tensor("olt", [64, LOCAL], BF16, kind="ExternalInput")
    yt = nc.dram_tensor("yt", [64, LOCAL], F32, kind="ExternalOutput")

    # Engine assignment: most tiles ACT-evac + DVE-compare ("A"); n_d tiles
    # are compared straight from PSUM on DVE, unmerged -> 2x yT ("D").
    # Local blocks (b < nloc) need a single merged mask for affine_select,
    # so they never take the D path.
    n_d = n_kb - n_a - n_p  # back-compat: n_a + n_p = number of A tiles
    assign = ["A"] * n_kb
    if n_d > 0:
        stride = max((n_kb - nloc) // n_d, 1)
        placed = 0
        b = nloc
        while placed < n_d and b < n_kb:
            assign[b] = "D"
            placed += 1
            b += stride

    if mask_bufs is None:
        mask_bufs = 2 * lag + 4
    with tile.TileContext(nc) as tc:
        with (
            tc.tile_pool(name="big", bufs=1) as big,
            tc.tile_pool(name="maskp", bufs=mask_bufs) as maskp,
            tc.tile_pool(name="absp", bufs=6) as absp,
            tc.tile_pool(name="outp", bufs=2) as outp,
            tc.tile_pool(name="fidp", bufs=fid_bufs, space="PSUM") as fidp,
            tc.tile_pool(name="ytp", bufs=1, space="PSUM") as ytp,
            (tc.For_i(0, loop_reps) if loop_reps > 1
             else contextlib.nullcontext()),
        ):
            for rep in range(reps):
                nf_sb = big.tile([128, ngrp, 2, 128], FP8, tag="nf")
                nl_sb = big.tile([128, 2, LOCAL], FP8, tag="nl")
                ob_sb = big.tile([128, n_kb, 64], BF16, tag="ob")
                ol_sb = big.tile([64, LOCAL], BF16, tag="ol")
                # critical-path order: local cols + first nf groups so fid
                # starts ASAP; first ob blocks before the yT lag expires
                nc.sync.dma_start(nl_sb[:], nl8[:])
                nc.sync.dma_start(nf_sb[:, 0:4], nf8[:, 0:4])
                nc.sync.dma_start(ob_sb[:, 0:16], obf[:, 0:16, :])
                nc.gpsimd.dma_start(nf_sb[:, 4:ngrp], nf8[:, 4:ngrp])
                nc.gpsimd.dma_start(ob_sb[:, 16:n_kb], obf[:, 16:n_kb, :])
                nc.gpsimd.dma_start(ol_sb[:], olt[:])

                yps = ytp.tile([128, 512], F32, tag="y")
                # constant threshold tensors (native tensor_tensor compares;
                # tensor_scalar is software-lowered and ~20x slower)
                if rep == 0:
                    thp = big.tile([128, 1024], BF16, tag="thp")
                    thn = big.tile([128, 1024], BF16, tag="thn")
                    nc.vector.memset(thp[:], SQRT_T)
                    nc.vector.memset(thn[:], -SQRT_T)
                if mode in ("fidonly", "noew", "fidew", "fidcopy", "fidabs"):
                    ds = outp.tile([128, n_kb], F32, tag="ds")
                if mode in ("noew", "ytonly"):
                    cmask = maskp.tile([128, 1024], BF16, tag="cm")
                    nc.vector.memset(cmask[:], 0.0)


                # Software-pipelined stages (per step s):
                #   yT(s-lag) first (deps oldest), fid(s), evac(s-1),
                #   A-compare(s-1-clag).  Staging keeps every engine queue's
                #   head dependency-free (avoids head-of-line stalls).
                clag = 3
                mq = {}
                fq = {}
                aq = {}
                nlast = 0
                for step in range(n_kb + lag):
                    # ---- yT consumption, lag behind ----
                    if lag <= step < n_kb + lag and mode not in (
                            "fidonly", "fidew", "fidcopy", "fidabs"):
                        b2 = step - lag
                        parts = (cmask,) if mode == "ytonly" else mq.pop(b2)
                        for gi, gt in enumerate(parts):
                            first = (b2 == 0) and gi == 0
                            last = (b2 == n_kb - 1) and gi == len(parts) - 1
                            nc.tensor.matmul(
                                yps[0:64, :], ob_sb[:, b2, :],
                                gt[:, 0:512],
                                start=first, stop=last,
                                tile_position=(0, 0))
                            nc.tensor.matmul(
                                yps[64:128, :], ob_sb[:, b2, :],
                                gt[:, 512:1024],
                                start=first, stop=last,
                                tile_position=(0, 64))
                    # ---- fid gram matmuls ----
                    if step < n_kb and mode != "ytonly":
                        b = step
                        g, t = b // 4, b % 4
                        sl32 = slice(32 * t, 32 * t + 32)
                        fps = fidp.tile([128, 1024], F32, tag="fid")
                        for half in range(2):
                            nsl = slice(half * 512, (half + 1) * 512)
                            nc.tensor.matmul(
                                fps[:, nsl], nf_sb[sl32, g, :, :],
                                nl_sb[sl32, :, nsl],
                                start=True, stop=True,
                                perf_mode=PM.DoubleRow,
                                tile_position=(32 * t, 0))
                        fq[b] = fps
                        if mode in ("fidonly", "noew"):
                            nc.vector.tensor_copy(ds[:, b:b + 1], fps[:, 0:1])
                            if mode == "noew":
                                mq[b] = (cmask,)
                            fq.pop(b)
                            continue
                        if mode in ("fidcopy", "fidabs"):
                            ab = absp.tile([128, 1024], BF16, tag="abs")
                            for half in range(2):
                                hs = slice(half * 512, (half + 1) * 512)
                                if mode == "fidcopy":
                                    nc.vector.tensor_copy(ab[:, hs],
                                                          fps[:, hs])
                                else:
                                    nc.scalar.activation(ab[:, hs],
                                                         fps[:, hs], AF.Abs)
                            nc.vector.tensor_copy(ds[:, b:b + 1], ab[:, 0:1])
                            fq.pop(b)
                            continue
                    if mode in ("fidonly", "noew", "fidcopy", "fidabs",
                                "ytonly"):
                        continue
                    # ---- evacuation stage, 1 behind fid ----
                    be = step - 1
                    if 0 <= be < n_kb:
                        fps_e = fq[be]
                        if assign[be] == "D":
                            # two one-sided compares straight from PSUM;
                            # left unmerged (yT consumes both: or == add)
                            g1 = maskp.tile([128, 1024], BF16, tag="mask")
                            g2 = maskp.tile([128, 1024], BF16, tag="mask")
                            nc.vector.tensor_tensor(
                                g1[:], fps_e[:], thp[:], op=ALU.is_ge)
                            nc.vector.tensor_tensor(
                                g2[:], fps_e[:], thn[:], op=ALU.is_le)
                            mq[be] = (g1, g2)
                            fq.pop(be)
                        else:
                            ab = absp.tile([128, 1024], BF16, tag="abs")
                            nc.scalar.activation(ab[:], fps_e[:], AF.Abs)
                            aq[be] = ab
                            fq.pop(be)
                    # ---- A-path compare stage, clag behind evac ----
                    bc = step - 1 - clag
                    if 0 <= bc < n_kb and assign[bc] == "A":
                        ab_c = aq.pop(bc)
                        mask = maskp.tile([128, 1024], BF16, tag="mask")
                        nc.vector.tensor_tensor(
                            mask[:], ab_c[:], thp[:], op=ALU.is_ge)
                        if bc < nloc and not no_affine:
                            # force the diagonal of local block bc to 1
                            # (iota = j - p - 128*bc; keep where != 0)
                            nc.gpsimd.affine_select(
                                mask[:], mask[:], pattern=[[1, 1024]],
                                compare_op=ALU.not_equal, fill=1.0,
                                base=-128 * bc, channel_multiplier=-1)
                        mq[bc] = (mask,)
                    if mode == "fidew" and 0 <= bc < n_kb:
                        for gi, gt in enumerate(mq.pop(bc)):
                            nc.vector.tensor_copy(
                                ds[:, bc:bc + 1], gt[:, gi:gi + 1])

                # yT -= out_local^T (removes the forced diagonal), then DMA
                if mode not in ("fidonly", "fidew", "fidcopy", "fidabs"):
                    for half in range(2):
                        jsl = slice(half * 512, (half + 1) * 512)
                        ysb = outp.tile([64, 512], F32, tag="ys")
                        nc.vector.tensor_tensor(
                            ysb[:], yps[half * 64:(half + 1) * 64, :],
                            ol_sb[:, jsl], op=ALU.subtract)
                        nc.sync.dma_start(yt[:, jsl], ysb[:])
                if mode in ("fidonly", "noew", "fidew", "fidcopy", "fidabs"):
                    nc.sync.dma_start(yt[:, 0:n_kb], ds[0:64, :])

    nc.compile()
    return nc


# ---------------------------------------------------------------------------
# Host orchestration
# ---------------------------------------------------------------------------
_CACHE = {}
LAST_RESULTS = {}


def _get(name, builder):
    if name not in _CACHE:
        nc = builder()
        nc.m = get_hw_module(nc.m)
        _CACHE[name] = nc
    return _CACHE[name]


def _phase1_inmaps(x, W1, b1, W2, b2, W3, b3):
    bf = ml_dtypes.bfloat16
    w1v = np.ascontiguousarray(
        W1.T.reshape(2, 128, 512).transpose(1, 0, 2)).reshape(128, 1024)
    w2v = np.ascontiguousarray(
        W2.T.reshape(4, 128, 256).transpose(1, 0, 2)).reshape(128, 1024)
    w3v = np.ascontiguousarray(
        W3.T.reshape(2, 128, 64).transpose(1, 0, 2)).reshape(128, 128)
    wpk = np.concatenate([w1v, w2v, w3v], axis=1).astype(bf)
    bpk = np.concatenate(
        [b1.reshape(4, 128).T, b2.reshape(2, 128).T], axis=1
    ).astype(np.float32)
    bpk = np.ascontiguousarray(bpk)
    maps = []
    for c in range(NCORES):
        xT = np.ascontiguousarray(
            x[c * LOCAL:(c + 1) * LOCAL].T.reshape(2, 128, LOCAL)
            .transpose(1, 0, 2)).astype(bf)
        maps.append(dict(xT=xT, wpk=wpk, bpk=bpk))
    return maps


def _phase2_inmaps(normedT_full, out_full):
    fp8 = ml_dtypes.float8_e4m3
    out_bf = out_full.astype(ml_dtypes.bfloat16)
    n_kb = B // 128
    ngrp = n_kb // 4
    maps = []
    for c in range(NCORES):
        nfull = np.roll(normedT_full, -LOCAL * c, axis=1)   # [64, 8192] f32
        # nf8[32t+ki, g, ko, m] = nfull[d = ki + 32*ko, row 128*(4g+t) + m]
        blk = nfull.reshape(2, 32, ngrp, 4, 128)            # [ko, ki, g, t, m]
        nf8 = np.ascontiguousarray(
            blk.transpose(3, 1, 2, 0, 4).reshape(128, ngrp, 2, 128)).astype(fp8)
        nloc = normedT_full[:, c * LOCAL:(c + 1) * LOCAL]   # [64, 1024] f32
        # nl8[32t+ki, ko, j] = nloc[ki + 32*ko, j], replicated over t
        nl1 = np.ascontiguousarray(
            nloc.reshape(2, 32, LOCAL).transpose(1, 0, 2))  # [ki, ko, j]
        nl8 = np.ascontiguousarray(
            np.broadcast_to(nl1, (4, 32, 2, LOCAL)).reshape(128, 2, LOCAL)
        ).astype(fp8)
        ob = np.roll(out_bf, -LOCAL * c, axis=0)
        olt = np.ascontiguousarray(ob[0:LOCAL].T)           # [64, 1024] bf16
        ob = np.ascontiguousarray(ob.reshape(64, 128, 64).transpose(1, 0, 2))
        maps.append(dict(nf8=nf8, nl8=nl8, obf=ob, olt=olt))
    return maps


def kernel(x, W1, b1, W2, b2, W3, b3, _trace=False):
    x, W1, b1, W2, b2, W3, b3 = [
        np.asarray(a, dtype=np.float32) for a in (x, W1, b1, W2, b2, W3, b3)]
    nc1 = _get("p1", build_phase1)
    nc2 = _get("p2", build_phase2)

    r1 = run_bass_kernel_spmd(nc1, _phase1_inmaps(x, W1, b1, W2, b2, W3, b3),
                              core_ids=list(range(NCORES)), trace=_trace)
    outT_full = np.concatenate([r1.results[c]["outT"] for c in range(NCORES)],
                               axis=1)
    outT_full = outT_full + b3[:, None].astype(np.float32)  # bias on host
    out_full = np.ascontiguousarray(outT_full.T)            # [8192, 64] f32
    norms = np.linalg.norm(out_full, axis=-1, keepdims=True)
    normed = out_full / (norms + np.float32(1e-12))
    normedT_full = np.ascontiguousarray(normed.T.astype(np.float32))

    r2 = run_bass_kernel_spmd(nc2, _phase2_inmaps(normedT_full, out_full),
                              core_ids=list(range(NCORES)), trace=_trace)
    y = np.concatenate(
        [np.ascontiguousarray(r2.results[c]["yt"].T) for c in range(NCORES)],
        axis=0)
    LAST_RESULTS["r1"] = r1
    LAST_RESULTS["r2"] = r2
    return y.astype(np.float32)
